# revision 36
# baseline (speedup 1.0000x reference)
"""Multi-head causal attention (B=2, T=2048, D=2048, H=16, dk=128) on 8 TRN2 NeuronCores.

Strategy (tensor-parallel over heads, 2 heads/core):
  - Host prep: transpose x -> xT [D, B*T], slice+transpose Wq/Wk/Wv per core
    ([D, 256] each), full Wo.T. All fed as float32 (device treats as f32r).
  - Per core: QT/KT = W.T-slices^T @ ... computed as PE matmuls producing
    Q^T/K^T layouts [dk, tokens]; V natural [tokens, dk].
  - Attention with TRANSPOSED scores: S^T[kk, q] chunks per kk-tile so the
    exp'd probabilities land directly in P^T layout (no PE transposes of P).
    No max-subtraction (scores are ~N(0,1); exp cannot overflow). Softmax
    denominator via ones[128,128] matmul over P^T partitions (result arrives
    pre-broadcast to all partitions); 1/d via DVE reciprocal_approx_fast,
    applied in the ctx PSUM->SBUF copy.
  - ctx^T [dk, tokens] per head -> AllToAll (2 MB/core) so each core gets all
    16 heads' ctx^T for its 512-token slice -> local Wo projection ->
    out [512, 2048]. Wo accumulation is split even/odd c-tiles: evens (ready
    after the first AllToAll) run during the second collective, partials spill
    to SBUF, odds accumulate in fresh PSUM, DVE add merges on the way out.
  - Host: concatenate the 8 row-slices.
  - Engine-queue placement is deliberate: collectives + cm loads on gpsimd,
    csb stores + half the woT stream on sync, exps + other half on scalar —
    a slot-waiting prefetch DMA must never sit ahead of critical work in an
    engine FIFO.

Everything matmul-facing uses dtype float32r: full TensorE rate (1 cyc/row,
same as bf16) at ~13-bit mantissa accuracy (~1.5e-4 per matmul).
"""

import math
import numpy as np
from contextlib import ExitStack

import concourse.tile as tile
import concourse.mybir as mybir
from concourse import bacc
from concourse.bass_utils import run_bass_kernel_spmd

B, T, D = 2, 2048, 2048
H, DK = 16, 128
NCORES = 8
HL = H // NCORES            # 2 heads per core
OC = HL * DK                # 256 out dims per core
TT = B * T                  # 4096 flat tokens
TCHUNK = 512
NTC = TT // TCHUNK          # 8 token chunks (proj)
NKT = D // 128              # 16 contraction tiles
NQC = T // TCHUNK           # 4 q-chunks per batch
SCALE = 1.0 / math.sqrt(DK)
F32 = mybir.dt.float32
MDT = mybir.dt.float32r
MASK_VAL = -1e30

_CACHE = {}


def build():
    nc = bacc.Bacc("TRN2", target_bir_lowering=False, debug=False, num_devices=NCORES)

    xt_d = nc.dram_tensor("xt", [D, TT], MDT, kind="ExternalInput")
    wqt_d = nc.dram_tensor("wqt", [D, OC], MDT, kind="ExternalInput")
    wkt_d = nc.dram_tensor("wkt", [D, OC], MDT, kind="ExternalInput")
    wvt_d = nc.dram_tensor("wvt", [D, OC], MDT, kind="ExternalInput")
    wot_d = nc.dram_tensor("wot", [D, D], MDT, kind="ExternalInput")
    out_d = nc.dram_tensor("out", [TT // NCORES, D], F32, kind="ExternalOutput")

    with tile.TileContext(nc) as tc, ExitStack() as ctx:
        psum = ctx.enter_context(tc.tile_pool(name="ps", bufs=8, space="PSUM"))
        dram = ctx.enter_context(tc.tile_pool(name="dram", bufs=1, space="DRAM"))
        persist = ctx.enter_context(tc.tile_pool(name="persist", bufs=1))
        small = ctx.enter_context(tc.tile_pool(name="small", bufs=2))

        # ---- persistent SBUF: QT/KT [128, HL*TT] (cols: h*TT + flat_tok), V [128, 32*256]
        QT = persist.tile([128, HL * TT], MDT, name="QTs")
        KT = persist.tile([128, HL * TT], MDT, name="KTs")
        Vs = persist.tile([128, (TT // 128) * OC], MDT, name="Vs")

        # ---- identity (needed by proj-phase V transposes)
        ident = persist.tile([128, 128], MDT, name="ident")
        with tc.tile_pool(name="cstage0", bufs=1) as cstage0:
            ident_f = cstage0.tile([128, 128], F32, name="ident_f")
            from concourse.masks import make_identity
            make_identity(nc, ident_f[:])
            nc.vector.tensor_copy(ident[:], ident_f[:])

        # =================== Phase 1: QKV projections ===================
        with tc.tile_pool(name="proj", bufs=1) as projp, \
             tc.tile_pool(name="xtp", bufs=8) as xtp:
            wq_sb = projp.tile([128, NKT * OC], MDT, name="wq_sb")
            wk_sb = projp.tile([128, NKT * OC], MDT, name="wk_sb")
            wv_sb = projp.tile([128, NKT * OC], MDT, name="wv_sb")
            # split weight loads into k-groups so the first matmuls start early
            for kg in range(0, NKT, 4):
                for w_sb, w_d in ((wq_sb, wqt_d), (wk_sb, wkt_d), (wv_sb, wvt_d)):
                    nc.sync.dma_start(
                        out=w_sb[:, kg * OC:(kg + 4) * OC].rearrange("p (kt o) -> p kt o", kt=4),
                        in_=w_d.ap()[kg * 128:(kg + 4) * 128, :].rearrange("(kt p) o -> p kt o", p=128),
                    )

            for tcx in range(NTC):
                xts = []
                for k in range(NKT):
                    xt = xtp.tile([128, TCHUNK], MDT, tag="xt", name=f"xt_{tcx}_{k}")
                    nc.gpsimd.dma_start(
                        out=xt[:],
                        in_=xt_d.ap()[k * 128:(k + 1) * 128, tcx * TCHUNK:(tcx + 1) * TCHUNK],
                    )
                    xts.append(xt)

                qp = [psum.tile([128, 512], F32, tag="mm", name=f"qp{tcx}_{o}") for o in range(HL)]
                kp = [psum.tile([128, 512], F32, tag="mm", name=f"kp{tcx}_{o}") for o in range(HL)]
                vp = [psum.tile([128, 512], F32, tag="mm", name=f"vp{tcx}_{o}") for o in range(HL)]
                for k in range(NKT):
                    st, sp = (k == 0), (k == NKT - 1)
                    for o in range(HL):
                        nc.tensor.matmul(qp[o][:], wq_sb[:, k * OC + o * 128: k * OC + (o + 1) * 128],
                                         xts[k][:], start=st, stop=sp)
                        nc.tensor.matmul(kp[o][:], wk_sb[:, k * OC + o * 128: k * OC + (o + 1) * 128],
                                         xts[k][:], start=st, stop=sp)
                        nc.tensor.matmul(vp[o][:], wv_sb[:, k * OC + o * 128: k * OC + (o + 1) * 128],
                                         xts[k][:], start=st, stop=sp)
                for o in range(HL):
                    dst = slice(o * TT + tcx * TCHUNK, o * TT + (tcx + 1) * TCHUNK)
                    nc.scalar.copy(QT[:, dst], qp[o][:])
                    nc.scalar.copy(KT[:, dst], kp[o][:])
                # V^T -> V via PE transposes (V^T psum -> sbuf -> transpose -> V)
                for o in range(HL):
                    vts = small.tile([128, 512], MDT, tag="vts", name=f"vts{tcx}_{o}")
                    nc.vector.tensor_copy(vts[:], vp[o][:])
                    for t4 in range(4):
                        t32 = tcx * 4 + t4
                        vtp = psum.tile([128, 128], MDT, tag="mm", name=f"vtp{tcx}_{o}_{t4}")
                        nc.tensor.transpose(vtp[:], vts[:, t4 * 128:(t4 + 1) * 128], ident[:])
                        nc.vector.tensor_copy(Vs[:, t32 * OC + o * 128: t32 * OC + (o + 1) * 128], vtp[:])

        # ---- attention constants (emitted after proj so the first xt DMAs lead)
        maskT = persist.tile([128, 128], F32, name="maskT")
        nc.gpsimd.memset(maskT[:], 0.0)
        # keep 0 where q >= kk (predicate -x + y >= 0), else MASK_VAL
        nc.gpsimd.affine_select(
            out=maskT[:], in_=maskT[:], compare_op=mybir.AluOpType.is_ge,
            fill=MASK_VAL, base=0, pattern=[[1, 128]], channel_multiplier=-1,
        )
        onesk = persist.tile([128, 128], MDT, name="onesk")
        zeros = persist.tile([128, 128], MDT, name="zeros")
        with tc.tile_pool(name="cstage", bufs=1) as cstage:
            ones_f = cstage.tile([128, 128], F32, name="ones_f")
            nc.gpsimd.memset(ones_f[:], 1.0)
            nc.vector.tensor_copy(onesk[:], ones_f[:])
            zeros_f = cstage.tile([128, 128], F32, name="zeros_f")
            nc.gpsimd.memset(zeros_f[:], 0.0)
            nc.vector.tensor_copy(zeros[:], zeros_f[:])


        # =================== Phase 2: attention ===================
        a2a_in = [dram.tile([NCORES, 128, TCHUNK], MDT, name=f"a2a_in{h}") for h in range(HL)]
        a2a_out = [dram.tile([NCORES, 128, TCHUNK], MDT, name=f"a2a_out{h}")
                   for h in range(HL)]

        # wo-phase pools opened BEFORE the attention pool so woT prefetch and
        # the hl=0 ctx loads can run concurrently with attention compute.
        wope = ctx.enter_context(tc.tile_pool(name="wope", bufs=1))
        wotp = ctx.enter_context(tc.tile_pool(name="wotp", bufs=4))
        cm = [None] * (2 * NCORES)
        c_order = [2 * i for i in range(NCORES)] + [2 * i + 1 for i in range(NCORES)]
        wts = {}

        with tc.tile_pool(name="ptp", bufs=2) as ptp:
            for hl in range(HL):
                for J in reversed(range(NQC)):
                    for b in range(B):
                        base = hl * TT + b * T
                        nkk = 4 * J + 4
                        ptiles = []
                        for kk in range(nkk):
                            pt = ptp.tile([128, 512], MDT, tag=f"pt{kk}", name=f"p_{hl}{b}{J}_{kk}", bufs=2 if kk < 8 else 1)
                            ptiles.append(pt)
                            s_off = max(0, (kk - 4 * J) * 128)
                            npr = 512 - s_off
                            st = psum.tile([128, 512], F32, tag="mm", name=f"st{hl}{b}{J}_{kk}")
                            nc.tensor.matmul(
                                st[:, :npr],
                                KT[:, base + kk * 128: base + (kk + 1) * 128],
                                QT[:, base + J * 512 + s_off: base + (J + 1) * 512],
                                start=True, stop=True,
                            )
                            if kk >= 4 * J:  # diagonal tile: causal mask
                                nc.vector.tensor_add(st[:, 0:128], st[:, 0:128], maskT[:])
                            nc.scalar.activation(pt[:, s_off:512], st[:, :npr],
                                                 mybir.ActivationFunctionType.Exp, scale=SCALE)
                            for zoff in range(0, s_off, 128):
                                nc.vector.tensor_copy(pt[:, zoff:zoff + 128], zeros[:])
                        # denominator (broadcast to all 128 partitions): d[p, q] = sum_kk P^T
                        dp = psum.tile([128, 512], F32, tag="mm", name=f"dp{hl}{b}{J}")
                        for kk in range(nkk):
                            nc.tensor.matmul(dp[:], onesk[:], ptiles[kk][:],
                                             start=(kk == 0), stop=(kk == nkk - 1))
                        dsb = small.tile([128, 512], F32, tag="dsb", name=f"dsb_{hl}{b}{J}")
                        nc.scalar.copy(dsb[:], dp[:])
                        rd = small.tile([128, 512], F32, tag="rd", name=f"rd_{hl}{b}{J}")
                        nc.vector.reciprocal_approx_fast(rd[:], dsb[:])
                        # ctx^T accumulate over kk
                        cp = psum.tile([128, 512], F32, tag="mm", name=f"cp{hl}{b}{J}")
                        for kk in range(nkk):
                            nc.tensor.matmul(
                                cp[:],
                                Vs[:, (b * 16 + kk) * OC + hl * 128: (b * 16 + kk) * OC + (hl + 1) * 128],
                                ptiles[kk][:],
                                start=(kk == 0), stop=(kk == nkk - 1),
                            )
                        csb = small.tile([128, 512], MDT, tag="csb", name=f"csb{hl}{b}{J}", bufs=3)
                        nc.vector.tensor_mul(csb[:], cp[:], rd[:])
                        nc.sync.dma_start(out=a2a_in[hl][b * NQC + J], in_=csb[:])
                nc.gpsimd.collective_compute(
                    "AllToAll", mybir.AluOpType.bypass,
                    replica_groups=[list(range(NCORES))],
                    ins=[a2a_in[hl].opt()], outs=[a2a_out[hl].opt()],
                )
            # after both heads' attention: load hl=0 ctx slices on the scalar
            # (ACT) queue -- its exp backlog is done, and A2A0 finished long ago
            cme = wope.tile([128, NCORES * TCHUNK], MDT, name="cme")
            nc.gpsimd.dma_start(
                out=cme[:].rearrange("p (c t) -> p c t", c=NCORES),
                in_=a2a_out[0].rearrange("c p t -> p c t"),
            )

        # =================== Phase 3: output projection ===================
        # Evens (ready after A2A0) accumulate for ALL o4 groups while A2A1 is
        # in flight, spilling partials to SBUF; odds then accumulate into
        # fresh PSUM and a DVE add merges the halves on the way out.
        with tc.tile_pool(name="wopo", bufs=1) as wopo, \
             tc.tile_pool(name="accp", bufs=1) as accp, \
             tc.tile_pool(name="outp", bufs=3) as outp:
            cmo = wopo.tile([128, NCORES * TCHUNK], MDT, name="cmo")
            nc.gpsimd.dma_start(
                out=cmo[:].rearrange("p (c t) -> p c t", c=NCORES),
                in_=a2a_out[1].rearrange("c p t -> p c t"),
            )
            evens = c_order[:NCORES]
            odds = c_order[NCORES:]
            acc = {}
            for pi, (oa, ob) in enumerate(((0, 1), (2, 3))):
                for ci, c16 in enumerate(evens):
                    wt = wotp.tile([128, 1024], MDT, tag="wot", name=f"wte{pi}_{c16}")
                    eng = (nc.scalar, nc.sync)[ci % 2]
                    eng.dma_start(
                        out=wt[:],
                        in_=wot_d.ap()[c16 * 128:(c16 + 1) * 128, oa * 512:(ob + 1) * 512],
                    )
                    wts[(pi, c16)] = wt
                ops = {o4: [psum.tile([128, 512], F32, tag="mm", name=f"ope{o4}_{t}") for t in range(4)]
                       for o4 in (oa, ob)}
                for ci, c16 in enumerate(evens):
                    wt = wts[(pi, c16)]
                    i = c16 // 2
                    for oi, o4 in enumerate((oa, ob)):
                        for t4 in range(4):
                            nc.tensor.matmul(ops[o4][t4][:],
                                             cme[:, i * 512 + t4 * 128: i * 512 + (t4 + 1) * 128],
                                             wt[:, oi * 512:(oi + 1) * 512],
                                             start=(ci == 0), stop=(ci == NCORES - 1))
                for o4 in (oa, ob):
                    for t4 in range(4):
                        a_ = accp.tile([128, 512], F32, name=f"acc{o4}_{t4}")
                        nc.scalar.copy(a_[:], ops[o4][t4][:])
                        acc[(o4, t4)] = a_
            for pi, (oa, ob) in enumerate(((0, 1), (2, 3))):
                for ci, c16 in enumerate(odds):
                    wt = wotp.tile([128, 1024], MDT, tag="wot", name=f"wto{pi}_{c16}")
                    eng = (nc.scalar, nc.sync, nc.gpsimd)[ci % 3]
                    eng.dma_start(
                        out=wt[:],
                        in_=wot_d.ap()[c16 * 128:(c16 + 1) * 128, oa * 512:(ob + 1) * 512],
                    )
                    wts[("o", pi, c16)] = wt
                ops = {o4: [psum.tile([128, 512], F32, tag="mm", name=f"opo{o4}_{t}") for t in range(4)]
                       for o4 in (oa, ob)}
                for ci, c16 in enumerate(odds):
                    wt = wts[("o", pi, c16)]
                    i = c16 // 2
                    for oi, o4 in enumerate((oa, ob)):
                        for t4 in range(4):
                            nc.tensor.matmul(ops[o4][t4][:],
                                             cmo[:, i * 512 + t4 * 128: i * 512 + (t4 + 1) * 128],
                                             wt[:, oi * 512:(oi + 1) * 512],
                                             start=(ci == 0), stop=(ci == NCORES - 1))
                for o4 in (oa, ob):
                    for t4 in range(4):
                        ot = outp.tile([128, 512], F32, tag="ot", name=f"ot{o4}_{t4}")
                        nc.vector.tensor_add(ot[:], ops[o4][t4][:], acc[(o4, t4)][:])
                        nc.sync.dma_start(
                            out=out_d.ap()[t4 * 128:(t4 + 1) * 128, o4 * 512:(o4 + 1) * 512],
                            in_=ot[:],
                        )

    nc.compile()
    return nc


def get_nc():
    if "nc" not in _CACHE:
        _CACHE["nc"] = build()
    return _CACHE["nc"]


def make_in_maps(x, wq, wk, wv, wo):
    x = np.asarray(x, dtype=np.float32)
    xT = np.ascontiguousarray(x.reshape(TT, D).T)
    woT = np.ascontiguousarray(np.asarray(wo, np.float32).T)
    in_maps = []
    for i in range(NCORES):
        sl = slice(i * OC, (i + 1) * OC)
        in_maps.append({
            "xt": xT,
            "wqt": np.ascontiguousarray(np.asarray(wq, np.float32)[sl, :].T),
            "wkt": np.ascontiguousarray(np.asarray(wk, np.float32)[sl, :].T),
            "wvt": np.ascontiguousarray(np.asarray(wv, np.float32)[sl, :].T),
            "wot": woT,
        })
    return in_maps


def assemble(results):
    return np.concatenate([results[i]["out"] for i in range(NCORES)], axis=0).reshape(B, T, D)


def kernel(x, wq, wk, wv, wo):
    nc = get_nc()
    in_maps = make_in_maps(x, wq, wk, wv, wo)
    res = run_bass_kernel_spmd(nc, in_maps, list(range(NCORES)), trace=False)
    return assemble(res.results)


if __name__ == "__main__":
    rng = np.random.default_rng(0)
    s = 1.0 / math.sqrt(D)
    x = rng.standard_normal((B, T, D), dtype=np.float32)
    wq = (rng.standard_normal((D, D), dtype=np.float32) * s)
    wk = (rng.standard_normal((D, D), dtype=np.float32) * s)
    wv = (rng.standard_normal((D, D), dtype=np.float32) * s)
    wo = (rng.standard_normal((D, D), dtype=np.float32) * s)
    out = kernel(x, wq, wk, wv, wo)
    print("out", out.shape, out.dtype, np.abs(out).mean())


# revision 37
# speedup vs baseline: 1.0264x; 1.0264x over previous
"""Multi-head causal attention (B=2, T=2048, D=2048, H=16, dk=128) on 8 TRN2 NeuronCores.

Strategy (tensor-parallel over heads, 2 heads/core):
  - Host prep: transpose x -> xT [D, B*T], slice+transpose Wq/Wk/Wv per core
    ([D, 256] each), full Wo.T. All fed as float32 (device treats as f32r).
  - Per core: QT/KT = W.T-slices^T @ ... computed as PE matmuls producing
    Q^T/K^T layouts [dk, tokens]; V natural [tokens, dk].
  - Attention with TRANSPOSED scores: S^T[kk, q] chunks per kk-tile so the
    exp'd probabilities land directly in P^T layout (no PE transposes of P).
    No max-subtraction (scores are ~N(0,1); exp cannot overflow). Softmax
    denominator via ones[128,128] matmul over P^T partitions (result arrives
    pre-broadcast to all partitions); 1/d via DVE reciprocal_approx_fast,
    applied in the ctx PSUM->SBUF copy.
  - ctx^T [dk, tokens] per head -> AllToAll (2 MB/core) so each core gets all
    16 heads' ctx^T for its 512-token slice -> local Wo projection ->
    out [512, 2048]. Wo accumulation is split even/odd c-tiles: evens (ready
    after the first AllToAll) run during the second collective, partials spill
    to SBUF, odds accumulate in fresh PSUM, DVE add merges on the way out.
  - Host: concatenate the 8 row-slices.
  - Engine-queue placement is deliberate: collectives + cm loads on gpsimd,
    csb stores + half the woT stream on sync, exps + other half on scalar —
    a slot-waiting prefetch DMA must never sit ahead of critical work in an
    engine FIFO.

Everything matmul-facing uses dtype float32r: full TensorE rate (1 cyc/row,
same as bf16) at ~13-bit mantissa accuracy (~1.5e-4 per matmul).
"""

import math
import numpy as np
from contextlib import ExitStack

import concourse.tile as tile
import concourse.mybir as mybir
from concourse import bacc
from concourse.bass_utils import run_bass_kernel_spmd

B, T, D = 2, 2048, 2048
H, DK = 16, 128
NCORES = 8
HL = H // NCORES            # 2 heads per core
OC = HL * DK                # 256 out dims per core
TT = B * T                  # 4096 flat tokens
TCHUNK = 512
NTC = TT // TCHUNK          # 8 token chunks (proj)
NKT = D // 128              # 16 contraction tiles
NQC = T // TCHUNK           # 4 q-chunks per batch
SCALE = 1.0 / math.sqrt(DK)
F32 = mybir.dt.float32
MDT = mybir.dt.float32r
MASK_VAL = -1e30

_CACHE = {}


def build():
    nc = bacc.Bacc("TRN2", target_bir_lowering=False, debug=False, num_devices=NCORES)

    xt_d = nc.dram_tensor("xt", [D, TT], MDT, kind="ExternalInput")
    wqt_d = nc.dram_tensor("wqt", [D, OC], MDT, kind="ExternalInput")
    wkt_d = nc.dram_tensor("wkt", [D, OC], MDT, kind="ExternalInput")
    wvt_d = nc.dram_tensor("wvt", [D, OC], MDT, kind="ExternalInput")
    wot_d = nc.dram_tensor("wot", [D, D], MDT, kind="ExternalInput")
    out_d = nc.dram_tensor("out", [TT // NCORES, D], F32, kind="ExternalOutput")

    with tile.TileContext(nc) as tc, ExitStack() as ctx:
        psum = ctx.enter_context(tc.tile_pool(name="ps", bufs=8, space="PSUM"))
        dram = ctx.enter_context(tc.tile_pool(name="dram", bufs=1, space="DRAM"))
        persist = ctx.enter_context(tc.tile_pool(name="persist", bufs=1))
        small = ctx.enter_context(tc.tile_pool(name="small", bufs=2))

        # ---- persistent SBUF: QT/KT [128, HL*TT] (cols: h*TT + flat_tok), V [128, 32*256]
        QT = persist.tile([128, HL * TT], MDT, name="QTs")
        KT = persist.tile([128, HL * TT], MDT, name="KTs")
        Vs = persist.tile([128, (TT // 128) * OC], MDT, name="Vs")

        # ---- identity (needed by proj-phase V transposes)
        ident = persist.tile([128, 128], MDT, name="ident")
        with tc.tile_pool(name="cstage0", bufs=1) as cstage0:
            ident_f = cstage0.tile([128, 128], F32, name="ident_f")
            from concourse.masks import make_identity
            make_identity(nc, ident_f[:])
            nc.vector.tensor_copy(ident[:], ident_f[:])

        # =================== Phase 1: QKV projections ===================
        with tc.tile_pool(name="proj", bufs=1) as projp, \
             tc.tile_pool(name="xtp", bufs=6) as xtp:
            wq_sb = projp.tile([128, NKT * OC], MDT, name="wq_sb")
            wk_sb = projp.tile([128, NKT * OC], MDT, name="wk_sb")
            wv_sb = projp.tile([128, NKT * OC], MDT, name="wv_sb")
            # split weight loads into k-groups so the first matmuls start early
            for kg in range(0, NKT, 4):
                for w_sb, w_d in ((wq_sb, wqt_d), (wk_sb, wkt_d), (wv_sb, wvt_d)):
                    nc.sync.dma_start(
                        out=w_sb[:, kg * OC:(kg + 4) * OC].rearrange("p (kt o) -> p kt o", kt=4),
                        in_=w_d.ap()[kg * 128:(kg + 4) * 128, :].rearrange("(kt p) o -> p kt o", p=128),
                    )

            for tcx in range(NTC):
                xts = []
                for k in range(NKT):
                    xt = xtp.tile([128, TCHUNK], MDT, tag="xt", name=f"xt_{tcx}_{k}")
                    nc.gpsimd.dma_start(
                        out=xt[:],
                        in_=xt_d.ap()[k * 128:(k + 1) * 128, tcx * TCHUNK:(tcx + 1) * TCHUNK],
                    )
                    xts.append(xt)

                qp = [psum.tile([128, 512], F32, tag="mm", name=f"qp{tcx}_{o}") for o in range(HL)]
                kp = [psum.tile([128, 512], F32, tag="mm", name=f"kp{tcx}_{o}") for o in range(HL)]
                vp = [psum.tile([128, 512], F32, tag="mm", name=f"vp{tcx}_{o}") for o in range(HL)]
                for k in range(NKT):
                    st, sp = (k == 0), (k == NKT - 1)
                    for o in range(HL):
                        nc.tensor.matmul(qp[o][:], wq_sb[:, k * OC + o * 128: k * OC + (o + 1) * 128],
                                         xts[k][:], start=st, stop=sp)
                        nc.tensor.matmul(kp[o][:], wk_sb[:, k * OC + o * 128: k * OC + (o + 1) * 128],
                                         xts[k][:], start=st, stop=sp)
                        nc.tensor.matmul(vp[o][:], wv_sb[:, k * OC + o * 128: k * OC + (o + 1) * 128],
                                         xts[k][:], start=st, stop=sp)
                for o in range(HL):
                    dst = slice(o * TT + tcx * TCHUNK, o * TT + (tcx + 1) * TCHUNK)
                    nc.scalar.copy(QT[:, dst], qp[o][:])
                    nc.scalar.copy(KT[:, dst], kp[o][:])
                # V^T -> V via PE transposes (V^T psum -> sbuf -> transpose -> V)
                for o in range(HL):
                    vts = small.tile([128, 512], MDT, tag="vts", name=f"vts{tcx}_{o}")
                    nc.vector.tensor_copy(vts[:], vp[o][:])
                    for t4 in range(4):
                        t32 = tcx * 4 + t4
                        vtp = psum.tile([128, 128], MDT, tag="mm", name=f"vtp{tcx}_{o}_{t4}")
                        nc.tensor.transpose(vtp[:], vts[:, t4 * 128:(t4 + 1) * 128], ident[:])
                        nc.vector.tensor_copy(Vs[:, t32 * OC + o * 128: t32 * OC + (o + 1) * 128], vtp[:])

        # ---- attention constants (emitted after proj so the first xt DMAs lead)
        maskT = persist.tile([128, 128], F32, name="maskT")
        nc.gpsimd.memset(maskT[:], 0.0)
        # keep 0 where q >= kk (predicate -x + y >= 0), else MASK_VAL
        nc.gpsimd.affine_select(
            out=maskT[:], in_=maskT[:], compare_op=mybir.AluOpType.is_ge,
            fill=MASK_VAL, base=0, pattern=[[1, 128]], channel_multiplier=-1,
        )
        onesk = persist.tile([128, 128], MDT, name="onesk")
        zeros = persist.tile([128, 128], MDT, name="zeros")
        with tc.tile_pool(name="cstage", bufs=1) as cstage:
            ones_f = cstage.tile([128, 128], F32, name="ones_f")
            nc.gpsimd.memset(ones_f[:], 1.0)
            nc.vector.tensor_copy(onesk[:], ones_f[:])
            zeros_f = cstage.tile([128, 128], F32, name="zeros_f")
            nc.gpsimd.memset(zeros_f[:], 0.0)
            nc.vector.tensor_copy(zeros[:], zeros_f[:])


        # =================== Phase 2: attention ===================
        a2a_in = [dram.tile([NCORES, 128, TCHUNK], MDT, name=f"a2a_in{h}") for h in range(HL)]
        a2a_out = [dram.tile([NCORES, 128, TCHUNK], MDT, name=f"a2a_out{h}")
                   for h in range(HL)]

        # wo-phase pools opened BEFORE the attention pool so woT prefetch and
        # the hl=0 ctx loads can run concurrently with attention compute.
        wope = ctx.enter_context(tc.tile_pool(name="wope", bufs=1))
        wotp = ctx.enter_context(tc.tile_pool(name="wotp", bufs=4))
        cm = [None] * (2 * NCORES)
        c_order = [2 * i for i in range(NCORES)] + [2 * i + 1 for i in range(NCORES)]
        wts = {}

        with tc.tile_pool(name="ptp", bufs=2) as ptp:
            for hl in range(HL):
                for J in reversed(range(NQC)):
                    for b in range(B):
                        base = hl * TT + b * T
                        nkk = 4 * J + 4
                        ptiles = []
                        for kk in range(nkk):
                            pt = ptp.tile([128, 512], MDT, tag=f"pt{kk}", name=f"p_{hl}{b}{J}_{kk}", bufs=2 if kk < 8 else 1)
                            ptiles.append(pt)
                            s_off = max(0, (kk - 4 * J) * 128)
                            npr = 512 - s_off
                            st = psum.tile([128, 512], F32, tag="mm", name=f"st{hl}{b}{J}_{kk}")
                            nc.tensor.matmul(
                                st[:, :npr],
                                KT[:, base + kk * 128: base + (kk + 1) * 128],
                                QT[:, base + J * 512 + s_off: base + (J + 1) * 512],
                                start=True, stop=True,
                            )
                            if kk >= 4 * J:  # diagonal tile: causal mask
                                nc.vector.tensor_add(st[:, 0:128], st[:, 0:128], maskT[:])
                            nc.scalar.activation(pt[:, s_off:512], st[:, :npr],
                                                 mybir.ActivationFunctionType.Exp, scale=SCALE)
                            for zoff in range(0, s_off, 128):
                                nc.vector.tensor_copy(pt[:, zoff:zoff + 128], zeros[:])
                        # denominator (broadcast to all 128 partitions): d[p, q] = sum_kk P^T
                        dp = psum.tile([128, 512], F32, tag="mm", name=f"dp{hl}{b}{J}")
                        for kk in range(nkk):
                            nc.tensor.matmul(dp[:], onesk[:], ptiles[kk][:],
                                             start=(kk == 0), stop=(kk == nkk - 1))
                        dsb = small.tile([128, 512], F32, tag="dsb", name=f"dsb_{hl}{b}{J}")
                        nc.scalar.copy(dsb[:], dp[:])
                        rd = small.tile([128, 512], F32, tag="rd", name=f"rd_{hl}{b}{J}")
                        nc.vector.reciprocal_approx_fast(rd[:], dsb[:])
                        # ctx^T accumulate over kk
                        cp = psum.tile([128, 512], F32, tag="mm", name=f"cp{hl}{b}{J}")
                        for kk in range(nkk):
                            nc.tensor.matmul(
                                cp[:],
                                Vs[:, (b * 16 + kk) * OC + hl * 128: (b * 16 + kk) * OC + (hl + 1) * 128],
                                ptiles[kk][:],
                                start=(kk == 0), stop=(kk == nkk - 1),
                            )
                        csb = small.tile([128, 512], MDT, tag="csb", name=f"csb{hl}{b}{J}", bufs=3)
                        nc.vector.tensor_mul(csb[:], cp[:], rd[:])
                        nc.sync.dma_start(out=a2a_in[hl][b * NQC + J], in_=csb[:])
                nc.gpsimd.collective_compute(
                    "AllToAll", mybir.AluOpType.bypass,
                    replica_groups=[list(range(NCORES))],
                    ins=[a2a_in[hl].opt()], outs=[a2a_out[hl].opt()],
                )
            # after both heads' attention: load hl=0 ctx slices on the scalar
            # (ACT) queue -- its exp backlog is done, and A2A0 finished long ago
            cme = wope.tile([128, NCORES * TCHUNK], MDT, name="cme")
            nc.gpsimd.dma_start(
                out=cme[:].rearrange("p (c t) -> p c t", c=NCORES),
                in_=a2a_out[0].rearrange("c p t -> p c t"),
            )

        # =================== Phase 3: output projection ===================
        # Evens (ready after A2A0) accumulate for ALL o4 groups while A2A1 is
        # in flight, spilling partials to SBUF; odds then accumulate into
        # fresh PSUM and a DVE add merges the halves on the way out.
        with tc.tile_pool(name="wopo", bufs=1) as wopo, \
             tc.tile_pool(name="accp", bufs=1) as accp, \
             tc.tile_pool(name="outp", bufs=3) as outp:
            cmo = wopo.tile([128, NCORES * TCHUNK], MDT, name="cmo")
            nc.gpsimd.dma_start(
                out=cmo[:].rearrange("p (c t) -> p c t", c=NCORES),
                in_=a2a_out[1].rearrange("c p t -> p c t"),
            )
            evens = c_order[:NCORES]
            odds = c_order[NCORES:]
            acc = {}
            for pi, (oa, ob) in enumerate(((0, 1), (2, 3))):
                for ci, c16 in enumerate(evens):
                    wt = wotp.tile([128, 1024], MDT, tag="wot", name=f"wte{pi}_{c16}")
                    eng = (nc.scalar, nc.sync)[ci % 2]
                    eng.dma_start(
                        out=wt[:],
                        in_=wot_d.ap()[c16 * 128:(c16 + 1) * 128, oa * 512:(ob + 1) * 512],
                    )
                    wts[(pi, c16)] = wt
                ops = {o4: [psum.tile([128, 512], F32, tag="mm", name=f"ope{o4}_{t}") for t in range(4)]
                       for o4 in (oa, ob)}
                for ci, c16 in enumerate(evens):
                    wt = wts[(pi, c16)]
                    i = c16 // 2
                    for oi, o4 in enumerate((oa, ob)):
                        for t4 in range(4):
                            nc.tensor.matmul(ops[o4][t4][:],
                                             cme[:, i * 512 + t4 * 128: i * 512 + (t4 + 1) * 128],
                                             wt[:, oi * 512:(oi + 1) * 512],
                                             start=(ci == 0), stop=(ci == NCORES - 1))
                for o4 in (oa, ob):
                    for t4 in range(4):
                        a_ = accp.tile([128, 512], F32, name=f"acc{o4}_{t4}")
                        nc.scalar.copy(a_[:], ops[o4][t4][:])
                        acc[(o4, t4)] = a_
            for pi, (oa, ob) in enumerate(((0, 1), (2, 3))):
                for ci, c16 in enumerate(odds):
                    wt = wotp.tile([128, 1024], MDT, tag="wot", name=f"wto{pi}_{c16}")
                    eng = (nc.scalar, nc.sync, nc.gpsimd)[ci % 3]
                    eng.dma_start(
                        out=wt[:],
                        in_=wot_d.ap()[c16 * 128:(c16 + 1) * 128, oa * 512:(ob + 1) * 512],
                    )
                    wts[("o", pi, c16)] = wt
                ops = {o4: [psum.tile([128, 512], F32, tag="mm", name=f"opo{o4}_{t}") for t in range(4)]
                       for o4 in (oa, ob)}
                for ci, c16 in enumerate(odds):
                    wt = wts[("o", pi, c16)]
                    i = c16 // 2
                    for oi, o4 in enumerate((oa, ob)):
                        for t4 in range(4):
                            nc.tensor.matmul(ops[o4][t4][:],
                                             cmo[:, i * 512 + t4 * 128: i * 512 + (t4 + 1) * 128],
                                             wt[:, oi * 512:(oi + 1) * 512],
                                             start=(ci == 0), stop=(ci == NCORES - 1))
                for o4 in (oa, ob):
                    for t4 in range(4):
                        ot = outp.tile([128, 512], F32, tag="ot", name=f"ot{o4}_{t4}")
                        nc.vector.tensor_add(ot[:], ops[o4][t4][:], acc[(o4, t4)][:])
                        nc.sync.dma_start(
                            out=out_d.ap()[t4 * 128:(t4 + 1) * 128, o4 * 512:(o4 + 1) * 512],
                            in_=ot[:],
                        )

    nc.compile()
    return nc


def get_nc():
    if "nc" not in _CACHE:
        _CACHE["nc"] = build()
    return _CACHE["nc"]


def make_in_maps(x, wq, wk, wv, wo):
    x = np.asarray(x, dtype=np.float32)
    xT = np.ascontiguousarray(x.reshape(TT, D).T)
    woT = np.ascontiguousarray(np.asarray(wo, np.float32).T)
    in_maps = []
    for i in range(NCORES):
        sl = slice(i * OC, (i + 1) * OC)
        in_maps.append({
            "xt": xT,
            "wqt": np.ascontiguousarray(np.asarray(wq, np.float32)[sl, :].T),
            "wkt": np.ascontiguousarray(np.asarray(wk, np.float32)[sl, :].T),
            "wvt": np.ascontiguousarray(np.asarray(wv, np.float32)[sl, :].T),
            "wot": woT,
        })
    return in_maps


def assemble(results):
    return np.concatenate([results[i]["out"] for i in range(NCORES)], axis=0).reshape(B, T, D)


def kernel(x, wq, wk, wv, wo):
    nc = get_nc()
    in_maps = make_in_maps(x, wq, wk, wv, wo)
    res = run_bass_kernel_spmd(nc, in_maps, list(range(NCORES)), trace=False)
    return assemble(res.results)


if __name__ == "__main__":
    rng = np.random.default_rng(0)
    s = 1.0 / math.sqrt(D)
    x = rng.standard_normal((B, T, D), dtype=np.float32)
    wq = (rng.standard_normal((D, D), dtype=np.float32) * s)
    wk = (rng.standard_normal((D, D), dtype=np.float32) * s)
    wv = (rng.standard_normal((D, D), dtype=np.float32) * s)
    wo = (rng.standard_normal((D, D), dtype=np.float32) * s)
    out = kernel(x, wq, wk, wv, wo)
    print("out", out.shape, out.dtype, np.abs(out).mean())
